# revision 10
# baseline (speedup 1.0000x reference)
"""CRF forward-algorithm loss kernel for Trainium2 (8 NeuronCores, data-parallel over batch).

Math: the reference computes, per batch column b,
    r[b] = logsumexp_tag( alpha_L[b,:] + transition[END,:] ),  L = len[b]
where alpha follows the log-space recurrence
    alpha_{t+1}[next] = logsumexp_prev( alpha_t[prev] + transition[next,prev] ) + feat_t[next]
and the mask freezes alpha once t >= len[b].

We run the recurrence in exp space:  a_t = exp(alpha_t - CZ*t)  laid out as
[tag=64 partitions, b=128 free] per core, so one 64x64 matmul per step
(P = exp(transition) @ a) plus one elementwise multiply by exp(feat - CZ).
CZ is a fixed per-step log offset that keeps a_t in fp32 range (the per-step
growth of alpha concentrates tightly around log(T) + 1/2 = ~4.66; cumulative
drift over 512 steps has std ~3.7, far inside fp32's e^+-88 range).

Masking needs no per-step blending: for each b only the value at t = len[b] is
read.  Every step t >= TQ0 we also extract q_t = EE . a_t (EE = exp(transition[END,:]))
with a 1-column matmul into a PSUM row buffer; at the end
    r[b] = sum_t delta_t[b] * log q_t[b] + CZ*len[b]
where delta_t = m[t-1] - m[t] is a host-precomputed one-hot at t = len[b]
(lens are in [256,512], so rows t < TQ0 = 193 are skipped entirely).
The final sum over the 64-row q block buffer is a ones-vector matmul.
"""

import sys

import numpy as np

sys.path.insert(0, "/opt/trn_rl_repo")

S, B, T = 512, 1024, 64
NCORES = 8
BL = B // NCORES  # 128 batch columns per core
CZ = 4.667        # deterministic per-step log offset
TQ0 = 193         # first step with q extraction; 320 rows cover t in [193, 512]
NQB = 5           # q blocks of 64 rows each
BLK = 16          # feat steps per DMA/exp block

_cache: dict = {}
LAST_EXEC_NS = None


def _build():
    import concourse.bacc as bacc
    import concourse.bass as bass
    import concourse.mybir as mybir
    import concourse.tile as tile

    f32 = mybir.dt.float32
    AF = mybir.ActivationFunctionType

    nc = bacc.Bacc("TRN2", target_bir_lowering=False, debug=False, enable_asserts=False)

    feats_d = nc.dram_tensor("feats_t", (T, S, BL), f32, kind="ExternalInput")
    lt_d = nc.dram_tensor("ltrans", (T, T + 1), f32, kind="ExternalInput")
    ltq_d = nc.dram_tensor("ltq", (T, 64, 64), f32, kind="ExternalInput")
    delta_d = nc.dram_tensor("delta", (64, NQB, BL), f32, kind="ExternalInput")
    tw_d = nc.dram_tensor("tw", (1, BL), f32, kind="ExternalInput")
    out_d = nc.dram_tensor("out", (1, BL), f32, kind="ExternalOutput")

    with tile.TileContext(nc) as tc:
        with (
            tc.tile_pool(name="const", bufs=1) as cpool,
            tc.tile_pool(name="feat", bufs=3) as fpool,
            tc.tile_pool(name="ef", bufs=3) as efpool,
            tc.tile_pool(name="a", bufs=3) as apool,
            tc.tile_pool(name="acc", bufs=1) as accpool,
            tc.tile_pool(name="pp", bufs=4, space=bass.MemorySpace.PSUM) as ppool,
            tc.tile_pool(name="qp", bufs=2, space=bass.MemorySpace.PSUM) as qpool,
            tc.tile_pool(name="rp", bufs=1, space=bass.MemorySpace.PSUM) as rpool,
        ):
            bias0 = cpool.tile([T, 1], f32, tag="bias0")
            nc.vector.memset(bias0[:], 0.0)
            biasz = cpool.tile([T, 1], f32, tag="biasz")
            nc.vector.memset(biasz[:], -CZ)

            lt_log = cpool.tile([T, T + 1], f32, tag="lt_log")
            nc.sync.dma_start(lt_log[:], lt_d[:])
            lt = cpool.tile([T, T + 1], f32, tag="lt")
            nc.scalar.activation(lt[:], lt_log[:], AF.Exp, bias=bias0[:])
            # one-hot-column q-extraction weights: ltq[:, kk, :] has EE in col kk
            ltq = cpool.tile([T, 64, 64], f32, tag="ltq")
            nc.sync.dma_start(ltq[:], ltq_d[:])

            delta = cpool.tile([64, NQB, BL], f32, tag="delta")
            nc.sync.dma_start(delta[:], delta_d[:])
            tw = cpool.tile([1, BL], f32, tag="tw")
            nc.sync.dma_start(tw[:], tw_d[:])
            ones = cpool.tile([T, 1], f32, tag="ones")
            nc.vector.memset(ones[:], 1.0)

            qsave = accpool.tile([64, NQB, BL], f32, tag="qsave")

            a = apool.tile([T, BL], f32, tag="a")
            nc.vector.memset(a[:], 0.0)
            nc.vector.memset(a[0:1, :], 1.0)

            qblk = None
            for blk in range(S // BLK):
                t0 = blk * BLK
                fb = fpool.tile([T, BLK, BL], f32, tag="fb")
                nc.sync.dma_start(fb[:], feats_d[:, t0 : t0 + BLK, :])
                ef = efpool.tile([T, BLK, BL], f32, tag="ef")
                nc.scalar.activation(ef[:], fb[:], AF.Exp, bias=biasz[:])
                for k in range(BLK):
                    t = t0 + k
                    if t >= TQ0:
                        jj, kk = divmod(t - TQ0, 64)
                        if kk == 0:
                            qblk = qpool.tile([64, BL], f32, tag="qblk")
                        nc.tensor.matmul(
                            qblk[:], ltq[:, kk, :], a[:],
                            start=(kk == 0), stop=(kk == 63),
                            skip_group_check=True,
                        )
                        if kk == 63:
                            nc.vector.tensor_copy(qsave[:, jj, :], qblk[:])
                    p = ppool.tile([T, BL], f32, tag="p")
                    nc.tensor.matmul(p[:], lt[:, 0:T], a[:], start=True, stop=True)
                    anew = apool.tile([T, BL], f32, tag="a")
                    nc.vector.tensor_mul(anew[:], p[:], ef[:, k, :])
                    a = anew

            # q row for t = 512 (block 4, row 63), then flush block 4
            nc.tensor.matmul(
                qblk[:], ltq[:, 63, :], a[:],
                start=False, stop=True, skip_group_check=True,
            )
            nc.vector.tensor_copy(qsave[:, NQB - 1, :], qblk[:])

            logq = accpool.tile([64, NQB, BL], f32, tag="logq")
            nc.scalar.activation(logq[:], qsave[:], AF.Ln, bias=bias0[:])
            r1 = accpool.tile([64, NQB, BL], f32, tag="r1")
            nc.vector.tensor_mul(r1[:], logq[:], delta[:])

            rsum = rpool.tile([1, BL], f32, tag="rsum")
            for j in range(NQB):
                nc.tensor.matmul(
                    rsum[:], ones[:], r1[:, j, :],
                    start=(j == 0), stop=(j == NQB - 1),
                )
            rout = accpool.tile([1, BL], f32, tag="rout")
            nc.vector.tensor_add(rout[:], rsum[:], tw[:])
            nc.sync.dma_start(out_d[:], rout[:])

    nc.compile()
    return nc


def _prep_inputs(feats, mask, transition):
    feats = np.asarray(feats, dtype=np.float32)
    mask = np.asarray(mask, dtype=np.float32)
    transition = np.asarray(transition, dtype=np.float32)

    lens = mask.sum(axis=0)  # (B,)
    m_pad = np.concatenate([mask, np.zeros((1, B), np.float32)], axis=0)
    # delta[k, j, b] = m[t-1, b] - m[t, b],  t = TQ0 + 64*j + k
    tt = TQ0 + 64 * np.arange(NQB)[None, :] + np.arange(64)[:, None]  # [64, NQB]
    delta_full = m_pad[tt - 1, :] - m_pad[tt, :]  # [64, NQB, B]

    lt_log = np.concatenate([transition.T, transition[1:2, :].T], axis=1)
    lt_log = np.ascontiguousarray(lt_log, dtype=np.float32)

    # ltq[k, kk, m] = exp(transition[END, k]) * (m == kk)
    ee = np.exp(transition[1, :].astype(np.float64)).astype(np.float32)  # (64,)
    ltq = np.zeros((T, 64, 64), np.float32)
    idx = np.arange(64)
    ltq[:, idx, idx] = ee[:, None]

    in_maps = []
    for c in range(NCORES):
        sl = slice(c * BL, (c + 1) * BL)
        in_maps.append(
            {
                "feats_t": np.ascontiguousarray(feats[:, sl, :].transpose(2, 0, 1)),
                "ltrans": lt_log,
                "ltq": ltq,
                "delta": np.ascontiguousarray(delta_full[:, :, sl]),
                "tw": np.ascontiguousarray((CZ * lens[sl]).astype(np.float32)[None, :]),
            }
        )
    return in_maps


def kernel(feats, mask, transition, trace=False):
    global LAST_EXEC_NS
    if "nc" not in _cache:
        _cache["nc"] = _build()
    nc = _cache["nc"]

    in_maps = _prep_inputs(feats, mask, transition)

    from concourse.bass_utils import run_bass_kernel_spmd

    res = run_bass_kernel_spmd(nc, in_maps, core_ids=list(range(NCORES)), trace=trace)
    LAST_EXEC_NS = res.exec_time_ns
    out = np.concatenate([r["out"][0] for r in res.results], axis=0)
    return out.astype(np.float32)


# revision 11
# speedup vs baseline: 2.1884x; 2.1884x over previous
"""CRF forward-algorithm loss kernel for Trainium2 (8 NeuronCores, data-parallel over batch).

Math: the reference computes, per batch column b,
    r[b] = logsumexp_tag( alpha_L[b,:] + transition[END,:] ),  L = len[b]
where alpha follows the log-space recurrence
    alpha_{t+1}[next] = logsumexp_prev( alpha_t[prev] + transition[next,prev] ) + feat_t[next]
and the mask freezes alpha once t >= len[b].

We run the recurrence in exp space:  a_t = exp(alpha_t - CZ*t).  CZ is a fixed
per-step log offset that keeps a_t inside fp32 range (per-step growth of alpha
concentrates tightly around log(T) + 1/2 ~ 4.66; cumulative drift over 512
steps has std ~3.7, far inside fp32's e^+-88).

Per-core layout is "packed": 128 partitions = (batch-group g in {0,1}) x (64
tags), free dim = 64 batch columns within the group.  One block-diagonal
128x128 bf16 matmul per step computes P = E @ a for both groups (N=64 moving
columns), then one DVE multiply forms a_{t+1} = P * exp(feat - CZ).

Masking needs no per-step blending: only t = len[b] is ever read.  Each step
t >= TQ0, a second matmul with a one-hot-column weight slice accumulates
q_t = EE . a_t  (EE = exp(transition[END,:])) into row (g*64 + t%64) of a PSUM
block QP += (EE (x) e_row) @ a; rows not selected get += 0.  Blocks of 64 steps
are copied to SBUF, and at the end
    r[b] = sum_t delta_t[b] * log q_t[b] + CZ*len[b]
where delta_t = m[t-1] - m[t] is a host-precomputed one-hot at t = len[b]
(lens are in [256,512], so steps t < TQ0 = 193 skip extraction).  The final
sum over the 64 step-rows is a two-column ones matmul.
"""

import sys

import numpy as np

sys.path.insert(0, "/opt/trn_rl_repo")

S, B, T = 512, 1024, 64
NCORES = 8
BL = B // NCORES   # 128 batch columns per core
G = 2              # batch groups packed on partitions
BG = BL // G       # 64 batch columns per group
CZ = 4.667         # deterministic per-step log offset
TQ0 = 193          # first step with q extraction; 320 rows cover t in [193, 512]
NQB = 5            # q blocks of 64 steps each
BLK = 16           # feat steps per DMA/exp block

_cache: dict = {}
LAST_EXEC_NS = None


def _build():
    import concourse.bacc as bacc
    import concourse.bass as bass
    import concourse.mybir as mybir
    import concourse.tile as tile

    f32 = mybir.dt.float32
    bf16 = mybir.dt.bfloat16
    AF = mybir.ActivationFunctionType

    nc = bacc.Bacc("TRN2", target_bir_lowering=False, debug=False, enable_asserts=False)

    feats_d = nc.dram_tensor("feats_t", (G * T, S, BG), f32, kind="ExternalInput")
    lt2_d = nc.dram_tensor("lt2", (G * T, G * T), f32, kind="ExternalInput")
    ltq2_d = nc.dram_tensor("ltq2", (G * T, 64, G * 64), f32, kind="ExternalInput")
    delta_d = nc.dram_tensor("delta", (G * 64, NQB, BG), f32, kind="ExternalInput")
    tw_d = nc.dram_tensor("tw", (G, BG), f32, kind="ExternalInput")
    out_d = nc.dram_tensor("out", (G, BG), f32, kind="ExternalOutput")

    P128 = G * T  # 128

    with tile.TileContext(nc) as tc:
        with (
            tc.tile_pool(name="const", bufs=1) as cpool,
            tc.tile_pool(name="feat", bufs=3) as fpool,
            tc.tile_pool(name="ef", bufs=3) as efpool,
            tc.tile_pool(name="a", bufs=3) as apool,
            tc.tile_pool(name="acc", bufs=1) as accpool,
            tc.tile_pool(name="pp", bufs=4, space=bass.MemorySpace.PSUM) as ppool,
            tc.tile_pool(name="qp", bufs=2, space=bass.MemorySpace.PSUM) as qpool,
            tc.tile_pool(name="rp", bufs=1, space=bass.MemorySpace.PSUM) as rpool,
        ):
            bias0 = cpool.tile([P128, 1], f32, tag="bias0")
            nc.vector.memset(bias0[:], 0.0)
            biasz = cpool.tile([P128, 1], f32, tag="biasz")
            nc.vector.memset(biasz[:], -CZ)

            # block-diag transition weights (log-space in DRAM, exp'd to bf16 here)
            lt2_log = cpool.tile([P128, P128], f32, tag="lt2_log")
            nc.sync.dma_start(lt2_log[:], lt2_d[:])
            lt2 = cpool.tile([P128, P128], bf16, tag="lt2")
            nc.scalar.activation(lt2[:], lt2_log[:], AF.Exp, bias=bias0[:])

            # one-hot-column q-extraction weights: ltq2[:, kk, :] has EE in col g*64+kk
            ltq2_log = cpool.tile([P128, 64, G * 64], f32, tag="ltq2_log")
            nc.sync.dma_start(ltq2_log[:], ltq2_d[:])
            ltq2 = cpool.tile([P128, 64, G * 64], bf16, tag="ltq2")
            nc.scalar.activation(ltq2[:], ltq2_log[:], AF.Exp, bias=bias0[:])

            delta = cpool.tile([G * 64, NQB, BG], f32, tag="delta")
            nc.sync.dma_start(delta[:], delta_d[:])
            tw = cpool.tile([G, BG], f32, tag="tw")
            nc.sync.dma_start(tw[:], tw_d[:])
            # two-column group-sum weights: col g = indicator(partition in group g)
            onesg = cpool.tile([P128, G], f32, tag="onesg")
            nc.vector.memset(onesg[:], 0.0)
            nc.vector.memset(onesg[0:64, 0:1], 1.0)
            nc.vector.memset(onesg[64:128, 1:2], 1.0)

            qsave = accpool.tile([G * 64, NQB, BG], f32, tag="qsave")

            a = apool.tile([P128, BG], bf16, tag="a")
            nc.vector.memset(a[:], 0.0)
            nc.vector.memset(a[0:1, :], 1.0)
            nc.vector.memset(a[64:65, :], 1.0)

            qblk = None
            for blk in range(S // BLK):
                t0 = blk * BLK
                fb = fpool.tile([P128, BLK, BG], f32, tag="fb")
                nc.sync.dma_start(fb[:], feats_d[:, t0 : t0 + BLK, :])
                ef = efpool.tile([P128, BLK, BG], bf16, tag="ef")
                nc.scalar.activation(ef[:], fb[:], AF.Exp, bias=biasz[:])
                for k in range(BLK):
                    t = t0 + k
                    if t >= TQ0:
                        jj, kk = divmod(t - TQ0, 64)
                        if kk == 0:
                            qblk = qpool.tile([G * 64, BG], f32, tag="qblk")
                        nc.tensor.matmul(
                            qblk[:], ltq2[:, kk, :], a[:],
                            start=(kk == 0), stop=(kk == 63),
                            skip_group_check=True,
                        )
                        if kk == 63:
                            nc.vector.tensor_copy(qsave[:, jj, :], qblk[:])
                    p = ppool.tile([P128, BG], f32, tag="p")
                    nc.tensor.matmul(p[:], lt2[:], a[:], start=True, stop=True)
                    anew = apool.tile([P128, BG], bf16, tag="a")
                    nc.vector.tensor_mul(anew[:], p[:], ef[:, k, :])
                    a = anew

            # q row for t = 512 (block 4, row 63), then flush block 4
            nc.tensor.matmul(
                qblk[:], ltq2[:, 63, :], a[:],
                start=False, stop=True, skip_group_check=True,
            )
            nc.vector.tensor_copy(qsave[:, NQB - 1, :], qblk[:])

            logq = accpool.tile([G * 64, NQB, BG], f32, tag="logq")
            nc.scalar.activation(logq[:], qsave[:], AF.Ln, bias=bias0[:])
            r1 = accpool.tile([G * 64, NQB, BG], f32, tag="r1")
            nc.vector.tensor_mul(r1[:], logq[:], delta[:])

            rsum = rpool.tile([G, BG], f32, tag="rsum")
            for j in range(NQB):
                nc.tensor.matmul(
                    rsum[:], onesg[:], r1[:, j, :],
                    start=(j == 0), stop=(j == NQB - 1),
                )
            rout = accpool.tile([G, BG], f32, tag="rout")
            nc.vector.tensor_add(rout[:], rsum[:], tw[:])
            nc.sync.dma_start(out_d[:], rout[:])

    nc.compile()
    return nc


def _prep_inputs(feats, mask, transition):
    feats = np.asarray(feats, dtype=np.float32)
    mask = np.asarray(mask, dtype=np.float32)
    transition = np.asarray(transition, dtype=np.float32)

    lens = mask.sum(axis=0)  # (B,)
    m_pad = np.concatenate([mask, np.zeros((1, B), np.float32)], axis=0)
    # delta rows r = g*64 + kk, block j: t = TQ0 + 64*j + kk
    tt = TQ0 + 64 * np.arange(NQB)[None, :] + np.arange(64)[:, None]  # [64, NQB]
    delta_full = m_pad[tt - 1, :] - m_pad[tt, :]  # [64, NQB, B]

    NEG = -10000.0
    # block-diagonal log weights: lt2_log[g*64+p, g'*64+n] = trans[n,p] if g==g' else NEG
    lt2_log = np.full((G * T, G * T), NEG, np.float32)
    for g in range(G):
        lt2_log[g * T : (g + 1) * T, g * T : (g + 1) * T] = transition.T
    # one-hot q weights (log space): ltq2_log[g*64+p, kk, m] = trans[END,p] if m==g*64+kk
    ltq2_log = np.full((G * T, 64, G * 64), NEG, np.float32)
    idx = np.arange(64)
    for g in range(G):
        ltq2_log[g * T : (g + 1) * T, idx, g * 64 + idx] = transition[1, :][:, None]

    in_maps = []
    for c in range(NCORES):
        sl = slice(c * BL, (c + 1) * BL)
        fc = feats[:, sl, :]  # (S, BL, T)
        # packed layout [(g*64+tag), t, b']
        fp = np.ascontiguousarray(
            fc.reshape(S, G, BG, T).transpose(1, 3, 0, 2).reshape(G * T, S, BG)
        )
        dc = delta_full[:, :, sl]  # [64, NQB, BL]
        dpacked = np.ascontiguousarray(
            dc.reshape(64, NQB, G, BG).transpose(2, 0, 1, 3).reshape(G * 64, NQB, BG)
        )
        in_maps.append(
            {
                "feats_t": fp,
                "lt2": lt2_log,
                "ltq2": ltq2_log,
                "delta": dpacked,
                "tw": np.ascontiguousarray(
                    (CZ * lens[sl]).astype(np.float32).reshape(G, BG)
                ),
            }
        )
    return in_maps


def kernel(feats, mask, transition, trace=False):
    global LAST_EXEC_NS
    if "nc" not in _cache:
        _cache["nc"] = _build()
    nc = _cache["nc"]

    in_maps = _prep_inputs(feats, mask, transition)

    from concourse.bass_utils import run_bass_kernel_spmd

    res = run_bass_kernel_spmd(nc, in_maps, core_ids=list(range(NCORES)), trace=trace)
    LAST_EXEC_NS = res.exec_time_ns
    out = np.concatenate([r["out"].reshape(BL) for r in res.results], axis=0)
    return out.astype(np.float32)
